# revision 38
# baseline (speedup 1.0000x reference)
"""Trainium2 Bass kernel for EquivariantGraphConv message passing.

Math (reference):
    scalar = x[:,0,:]; vector = x[:,1:,:].reshape(N, 3H)
    scalar_out = scalar @ Wsr.T + b + segsum(scalar[col] @ Wsrel.T, row)
    vector_out = vector @ Wvr.T + segsum(vector[col] @ Wvrel.T, row)

Key identity: the edge transform is linear, so
    segsum(feat[col] @ W.T, row) == segsum(feat[col], row) @ W.T
We aggregate the raw 512-dim node features per destination first, then apply
all four weight matrices per node.

Sharding: destinations sharded across 8 cores (1280 nodes each in 10 chunks
of 128); edges sorted by destination on the host so no cross-core reduction.
Each core gathers source rows from a replicated node table in DRAM (indirect
DMA), builds one-hot "selection" matrices on the vector engine and
matmul-accumulates P^T @ G into PSUM to realize the segment sum.

v7: edges deduped per (chunk, source) into slots with up to L=3 destinations
(multi-hot one-hot layers = extra accumulate matmuls against the same G
tiles) — cuts gather traffic ~12%.  Chunks processed in descending slot-count
order per core; trailing gather padding uses -1 indices (skipped by the DMA
ucode) with a cross-core-constant valid count per piece.
"""

import os
import sys

sys.path.insert(0, "/opt/trn_rl_repo")

import numpy as np
import ml_dtypes

import concourse.bass as bass
import concourse.mybir as mybir
import concourse.tile as tile
from concourse.bacc import Bacc
from concourse.bass_utils import run_bass_kernel_spmd

N_NODES = 10000
N_EDGES = 160000
H = 128
F = 4 * H            # 512 features per node (scalar 128 + vector 384)
P = 128              # partitions
NP_PAD = 10240       # padded node count (80 chunks of 128)
N_CORES = 8
NODES_PER_CORE = NP_PAD // N_CORES       # 1280
CHUNKS_PER_CORE = NODES_PER_CORE // P    # 10
N_CHUNKS = NP_PAD // P                   # 80
ZERO_ROW = N_NODES                       # padded zero row used by pad slots
L = 2                                    # max dests per dedup slot (layers)
GQ = 1024                                # max idxs per dma_gather piece
WQ = GQ // 16
ZPAD_POSITIONS = 6   # first positions pad with ZERO_ROW (uninit SBUF guard)

# configuration: (gather/stage1 dtype, stage2 dtype)
CFG = os.environ.get("BASS_GNN_CFG", "bf16,bf16")
# output dtype: bf16 halves the writeback DMA; upcast to f32 on host
OUT_BF16 = os.environ.get("BASS_GNN_OUT", "bf16") == "bf16"

# test.py hooks
PROFILE = {"on": False, "trace_cores": None, "last": None}

_prog_cache = {}


def _dt(name):
    return {
        "bf16": mybir.dt.bfloat16,
        "f32": mybir.dt.float32,
        "f32r": mybir.dt.float32,
    }[name]


def _npdt(name):
    return {
        "bf16": ml_dtypes.bfloat16,
        "f32": np.float32,
        "f32r": np.float32,
    }[name]


# ---------------------------------------------------------------------------
# host-side dedup packing
# ---------------------------------------------------------------------------

def _build_chunk_slots(srcs, dsts):
    """Group one chunk's edges by source; slots carry up to L dests each,
    sorted by #dests desc so one-hot layer k>1 covers only a prefix."""
    order = np.argsort(srcs, kind="stable")
    s = srcs[order]; t = dsts[order]
    uniq, start, cnt = np.unique(s, return_index=True, return_counts=True)
    slots = []
    for u, st, c in zip(uniq, start, cnt):
        ds = t[st:st + c]
        for k in range(0, len(ds), L):
            slots.append((int(u), list(ds[k:k + L])))
    slots.sort(key=lambda x: -len(x[1]))
    return slots


def _pack(row, col):
    order = np.argsort(row, kind="stable")
    row_s = row[order]; col_s = col[order]
    bounds = np.searchsorted(row_s, np.arange(0, NP_PAD + 1, P))

    chunk_slots = []
    for g in range(N_CHUNKS):
        s, e = bounds[g], bounds[g + 1]
        chunk_slots.append(_build_chunk_slots(col_s[s:e], (row_s[s:e] - g * P)))

    perms = []
    for k in range(N_CORES):
        counts = [len(chunk_slots[k * CHUNKS_PER_CORE + j])
                  for j in range(CHUNKS_PER_CORE)]
        perms.append(sorted(range(CHUNKS_PER_CORE), key=lambda j: -counts[j]))

    M = np.zeros((N_CORES, CHUNKS_PER_CORE), dtype=int)
    N2 = np.zeros_like(M); N3 = np.zeros_like(M)
    for k in range(N_CORES):
        for i in range(CHUNKS_PER_CORE):
            sl = chunk_slots[k * CHUNKS_PER_CORE + perms[k][i]]
            M[k, i] = len(sl)
            N2[k, i] = sum(1 for _, d in sl if len(d) >= 2)
            N3[k, i] = sum(1 for _, d in sl if len(d) >= 3)
    Vmax = ((M.max(axis=0) + 15) // 16) * 16
    T1 = (Vmax + P - 1) // P
    T2 = (N2.max(axis=0) + P - 1) // P
    T3 = (N3.max(axis=0) + P - 1) // P
    NT = T1 + T2 + T3

    VQ = []
    for i in range(CHUNKS_PER_CORE):
        cap = int(T1[i]) * P
        v = cap if i < ZPAD_POSITIONS else int(Vmax[i])
        counts = []
        off = 0
        while off < cap:
            n = min(GQ, cap - off)
            counts.append(int(np.clip(v - off, 0, n)))
            off += n
        while counts and counts[-1] == 0:
            counts.pop()
        assert all(c >= 16 for c in counts), counts
        VQ.append(tuple(counts))

    NQ = [(int(T1[i]) * P + GQ - 1) // GQ for i in range(CHUNKS_PER_CORE)]
    cols_off = np.cumsum([0] + [NQ[i] * WQ for i in range(CHUNKS_PER_CORE)])
    rr_off = np.cumsum([0] + [int(NT[i]) for i in range(CHUNKS_PER_CORE)])
    cols_arr = np.zeros((N_CORES, P, int(cols_off[-1])), dtype=np.int16)
    rr_arr = np.full((N_CORES, P, int(rr_off[-1])), -1.0, dtype=np.float32)

    for k in range(N_CORES):
        for i in range(CHUNKS_PER_CORE):
            sl = chunk_slots[k * CHUNKS_PER_CORE + perms[k][i]]
            t1, t2, t3 = int(T1[i]), int(T2[i]), int(T3[i])
            cap = t1 * P
            m = len(sl)
            v = cap if i < ZPAD_POSITIONS else int(Vmax[i])
            flat = np.full(cap, -1, dtype=np.int16)
            flat[:m] = [u for u, _ in sl]
            flat[m:v] = ZERO_ROW
            co = int(cols_off[i])
            for q in range(len(VQ[i])):
                piece = np.full(GQ, -1, dtype=np.int16)
                seg = flat[q * GQ: min((q + 1) * GQ, cap)]
                piece[:len(seg)] = seg
                wrap = piece.reshape(WQ, 16).T
                cols_arr[k, :, co + q * WQ: co + (q + 1) * WQ] = np.tile(wrap, (8, 1))
            # dest layers
            dl = np.full((3, cap), -1.0, dtype=np.float32)
            for s_idx, (_, ds) in enumerate(sl):
                for layer in range(len(ds)):
                    dl[layer, s_idx] = float(ds[layer])
            ro = int(rr_off[i])
            rr_arr[k, :, ro:ro + t1] = dl[0].reshape(t1, P).T
            rr_arr[k, :, ro + t1:ro + t1 + t2] = dl[1, :t2 * P].reshape(t2, P).T
            rr_arr[k, :, ro + t1 + t2:ro + t1 + t2 + t3] = \
                dl[2, :t3 * P].reshape(t3, P).T

    meta = dict(T1=tuple(int(x) for x in T1), T2=tuple(int(x) for x in T2),
                T3=tuple(int(x) for x in T3), NT=tuple(int(x) for x in NT),
                VQ=tuple(VQ), NQ=tuple(NQ),
                cols_off=tuple(int(x) for x in cols_off),
                rr_off=tuple(int(x) for x in rr_off), perms=perms)
    return cols_arr, rr_arr, meta


# ---------------------------------------------------------------------------
# program builder
# ---------------------------------------------------------------------------

def _build_program(key):
    T1, T2, T3, NT, VQ, NQ, cols_off, rr_off, cfg = key
    s1_name, s2_name = cfg
    s1_store = _dt(s1_name)
    s2_store = _dt(s2_name)
    T1MAX = max(T1)
    NTMAX = max(NT)

    def s1(ap):
        return ap.bitcast(mybir.dt.float32r) if s1_name == "f32r" else ap

    def s2(ap):
        return ap.bitcast(mybir.dt.float32r) if s2_name == "f32r" else ap

    nc = Bacc("TRN2", num_swdge_queues=4)
    f32 = mybir.dt.float32

    xg = nc.dram_tensor("xg", [NP_PAD + 1, F], s1_store, kind="ExternalInput")
    cols = nc.dram_tensor("cols", [P, cols_off[-1]], mybir.dt.int16,
                          kind="ExternalInput")
    # rr/iota in bf16: exact for values in [-1, 127], 2x DVE is_equal rate
    rr = nc.dram_tensor("rr", [P, rr_off[-1]], s1_store, kind="ExternalInput")
    xt = nc.dram_tensor("xt", [P, 4 * NODES_PER_CORE], s2_store,
                        kind="ExternalInput")
    wsrel = nc.dram_tensor("wsrel", [P, H], s2_store, kind="ExternalInput")
    wsroot = nc.dram_tensor("wsroot", [P, H], s2_store, kind="ExternalInput")
    wvrel = nc.dram_tensor("wvrel", [P, 3 * 384], s2_store, kind="ExternalInput")
    wvroot = nc.dram_tensor("wvroot", [P, 3 * 384], s2_store, kind="ExternalInput")
    bias = nc.dram_tensor("bias", [P, H], f32, kind="ExternalInput")
    iota = nc.dram_tensor("iota", [P, P], s1_store, kind="ExternalInput")
    ident = nc.dram_tensor("ident", [P, P], s2_store, kind="ExternalInput")
    out_dt = mybir.dt.bfloat16 if OUT_BF16 else f32
    out = nc.dram_tensor("out", [NODES_PER_CORE, F], out_dt, kind="ExternalOutput")

    qctr = [0]

    with tile.TileContext(nc) as tc:
        with (
            tc.tile_pool(name="consts", bufs=1) as cpool,
            tc.tile_pool(name="edges", bufs=6) as epool,
            # gbuf bufs MUST equal ZPAD_POSITIONS: a chunk may read G slots
            # its gather skipped (-1 idx); those hold the previous cycle's
            # finite data only if its buffer was used ZPAD chunks earlier
            tc.tile_pool(name="gbuf", bufs=ZPAD_POSITIONS) as gpool,
            # agg/aggT staging buffered one per chunk: the DMA transposes
            # that read/write them starve behind the gather stream, so
            # buffer reuse would stall the pipeline (same for out staging)
            tc.tile_pool(name="work", bufs=CHUNKS_PER_CORE) as wpool,
            tc.tile_pool(name="outp", bufs=CHUNKS_PER_CORE) as opool,
            tc.tile_pool(name="pagg", bufs=4, space="PSUM") as pagg,
            tc.tile_pool(name="pmisc", bufs=2, space="PSUM") as pmisc,
        ):
            # iota is needed by the first is_equal — load it early (tiny)
            iota_sb = cpool.tile([P, P], s1_store)
            nc.scalar.dma_start(iota_sb[:], iota[:])

            LAG = 2
            agg_tiles = {}

            # per-chunk index DMAs, emitted before the big const loads so the
            # round-robin DMA sems recycle among small fast transfers
            cols_tiles = []
            for c in range(CHUNKS_PER_CORE):
                ct = cpool.tile([P, NQ[c] * WQ], mybir.dt.int16, tag=f"cols{c}")
                nc.sync.dma_start(ct[:], cols[:, cols_off[c]:cols_off[c + 1]])
                cols_tiles.append(ct)
            rr_all = cpool.tile([P, rr_off[-1]], s1_store)
            nc.sync.dma_start(rr_all[:], rr[:])

            # consts only feed stage2 (starts ~LAG chunks in); delay them so
            # the 2.6MB xt load doesn't steal DMA engines from early gathers
            with tc.tile_wait_until(0.012):
                xt_sb = cpool.tile([P, 4 * NODES_PER_CORE], s2_store)
                nc.scalar.dma_start(xt_sb[:], xt[:])
                wsrel_sb = cpool.tile([P, H], s2_store)
                nc.scalar.dma_start(wsrel_sb[:], wsrel[:])
                wsroot_sb = cpool.tile([P, H], s2_store)
                nc.scalar.dma_start(wsroot_sb[:], wsroot[:])
                wvrel_sb = cpool.tile([P, 3 * 384], s2_store)
                nc.scalar.dma_start(wvrel_sb[:], wvrel[:])
                wvroot_sb = cpool.tile([P, 3 * 384], s2_store)
                nc.scalar.dma_start(wvroot_sb[:], wvroot[:])
                bias_sb = cpool.tile([P, H], f32)
                nc.scalar.dma_start(bias_sb[:], bias[:])
                ident_sb = cpool.tile([P, P], s2_store)
                nc.scalar.dma_start(ident_sb[:], ident[:])

            def stage1(c):
                t1, t2, t3 = T1[c], T2[c], T3[c]
                nt = NT[c]
                cols_sb = cols_tiles[c]
                rr_sb = rr_all[:, rr_off[c]:rr_off[c] + nt]

                # gather: slot s -> G[s % 128, s // 128, :] = xg[cols[s], :]
                # pieces are (slot_start, nidx, valid) triples; the very first
                # gather is split small: its one-time ~9us cold prep would
                # otherwise block the other queues' preps on the Pool engine
                G = gpool.tile([P, T1MAX * F], s1_store, tag="G")
                pieces = []
                for q, vq in enumerate(VQ[c]):
                    start = q * GQ
                    nidx = min(GQ, t1 * P - start)
                    if c == 0 and q == 0:
                        pieces.append((0, 128, 128))
                        pieces.append((128, nidx - 128, max(0, vq - 128)))
                    else:
                        pieces.append((start, nidx, vq))
                for start, nidx, vq in pieces:
                    nslots = nidx // P
                    nc.gpsimd.dma_gather(
                        G[:, (start // P) * F:
                             (start // P + nslots) * F]
                        .rearrange("p (t f) -> p t f", f=F),
                        xg[:],
                        cols_sb[:, start // 16:start // 16 + (nidx + 15) // 16],
                        nidx,
                        vq,
                        F,
                        queue_num=qctr[0] % 4,
                    )
                    qctr[0] += 1

                # one-hot layers, all nt tiles in one DVE instruction
                Pm = epool.tile([P, NTMAX * P], s1_store, tag="P")
                nc.vector.tensor_tensor(
                    out=Pm[:, :nt * P].rearrange("p (t d) -> p t d", d=P),
                    in0=rr_sb.unsqueeze(2).to_broadcast([P, nt, P]),
                    in1=iota_sb[:].unsqueeze(1).to_broadcast([P, nt, P]),
                    op=mybir.AluOpType.is_equal,
                )

                # segment-sum: agg[d, f] += Pm_t^T @ G_t  (incl dedup layers)
                pairs = ([(j, j) for j in range(t1)]
                         + [(t1 + j, j) for j in range(t2)]
                         + [(t1 + t2 + j, j) for j in range(t3)])
                agg_ps = pagg.tile([P, F], f32, tag="agg")
                for n, (pm_t, g_t) in enumerate(pairs):
                    nc.tensor.matmul(
                        out=agg_ps[:],
                        lhsT=s1(Pm[:, pm_t * P:(pm_t + 1) * P]),
                        rhs=s1(G[:, g_t * F:(g_t + 1) * F]),
                        start=(n == 0),
                        stop=(n == len(pairs) - 1),
                    )
                agg_sb = wpool.tile([P, F], s2_store, tag="aggsb")
                nc.vector.tensor_copy(agg_sb[:], agg_ps[:])
                agg_tiles[c] = agg_sb

            def stage2(c):
                agg_sb = agg_tiles.pop(c)
                aggT_ps = pmisc.tile([P, F], s2_store, tag="aggT")
                for fc in range(4):
                    nc.tensor.transpose(
                        out=s2(aggT_ps[:, fc * P:(fc + 1) * P]),
                        in_=s2(agg_sb[:, fc * P:(fc + 1) * P]),
                        identity=s2(ident_sb[:]),
                    )
                aggT_sb = wpool.tile([P, F], s2_store, tag="aggTsb")
                nc.vector.tensor_copy(aggT_sb[:], aggT_ps[:])

                osv_ps = pmisc.tile([P, F], f32, tag="osv")
                nc.tensor.matmul(out=osv_ps[:, 0:H],
                                 lhsT=s2(aggT_sb[:, 0:P]), rhs=s2(wsrel_sb[:]),
                                 start=True, stop=False)
                nc.tensor.matmul(out=osv_ps[:, 0:H],
                                 lhsT=s2(xt_sb[:, c * P:(c + 1) * P]),
                                 rhs=s2(wsroot_sb[:]),
                                 start=False, stop=True)
                for kc in range(3):
                    nc.tensor.matmul(
                        out=osv_ps[:, H:F],
                        lhsT=s2(aggT_sb[:, (1 + kc) * P:(2 + kc) * P]),
                        rhs=s2(wvrel_sb[:, kc * 384:(kc + 1) * 384]),
                        start=(kc == 0), stop=False)
                for kc in range(3):
                    nc.tensor.matmul(
                        out=osv_ps[:, H:F],
                        lhsT=s2(xt_sb[:, (1 + kc) * NODES_PER_CORE + c * P:
                                      (1 + kc) * NODES_PER_CORE + (c + 1) * P]),
                        rhs=s2(wvroot_sb[:, kc * 384:(kc + 1) * 384]),
                        start=False, stop=(kc == 2))

                out_sb = opool.tile([P, F], out_dt, tag="outsb")
                nc.vector.tensor_add(out_sb[:, 0:H], osv_ps[:, 0:H], bias_sb[:])
                nc.vector.tensor_copy(out_sb[:, H:F], osv_ps[:, H:F])
                nc.sync.dma_start(out[c * P:(c + 1) * P, :], out_sb[:])

            for c in range(CHUNKS_PER_CORE + LAG):
                if c < CHUNKS_PER_CORE:
                    stage1(c)
                if c >= LAG:
                    stage2(c - LAG)

    nc.finalize()
    return nc


def _get_program(key):
    if key not in _prog_cache:
        _prog_cache[key] = _build_program(key)
    return _prog_cache[key]


def kernel(x, edge_index, W_scalar_rel, W_scalar_root, b_scalar_root,
           W_vector_rel, W_vector_root):
    cfg = tuple(CFG.split(","))
    s1_np = _npdt(cfg[0])
    s2_np = _npdt(cfg[1])

    x = np.asarray(x, dtype=np.float32)
    n = x.shape[0]
    assert n == N_NODES, x.shape
    row = np.asarray(edge_index[0], dtype=np.int64)
    col = np.asarray(edge_index[1], dtype=np.int64)

    cols_arr, rr_arr, meta = _pack(row, col)
    perms = meta["perms"]

    x_flat = np.zeros((NP_PAD + 1, F), dtype=np.float32)
    x_flat[:n] = x.reshape(n, F)
    xg_full = np.ascontiguousarray(x_flat.astype(s1_np))

    xT = x_flat[:NP_PAD].T  # [512, 10240]

    wsrelT = np.ascontiguousarray(np.asarray(W_scalar_rel, np.float32).T).astype(s2_np)
    wsrootT = np.ascontiguousarray(np.asarray(W_scalar_root, np.float32).T).astype(s2_np)
    wvrelT = np.ascontiguousarray(np.asarray(W_vector_rel, np.float32).T)
    wvrootT = np.ascontiguousarray(np.asarray(W_vector_root, np.float32).T)
    wvrel_packed = np.concatenate(
        [wvrelT[kc * P:(kc + 1) * P, :] for kc in range(3)], axis=1).astype(s2_np)
    wvroot_packed = np.concatenate(
        [wvrootT[kc * P:(kc + 1) * P, :] for kc in range(3)], axis=1).astype(s2_np)
    bias_t = np.ascontiguousarray(
        np.broadcast_to(np.asarray(b_scalar_root, np.float32), (P, H)))
    iota_t = np.ascontiguousarray(
        np.broadcast_to(np.arange(P, dtype=np.float32), (P, P))).astype(s1_np)
    ident_t = np.eye(P, dtype=np.float32).astype(s2_np)

    in_maps = []
    for core in range(N_CORES):
        base = core * NODES_PER_CORE
        # xt columns follow the chunk processing order of this core
        xTc = xT[:, base:base + NODES_PER_CORE]  # [512, 1280]
        xTp = np.concatenate(
            [xTc[:, perms[core][i] * P:(perms[core][i] + 1) * P]
             for i in range(CHUNKS_PER_CORE)], axis=1)
        xTr = np.ascontiguousarray(
            xTp.reshape(4, P, NODES_PER_CORE).transpose(1, 0, 2)
               .reshape(P, 4 * NODES_PER_CORE)).astype(s2_np)
        in_maps.append({
            "xg": xg_full,
            "cols": np.ascontiguousarray(cols_arr[core]),
            "rr": np.ascontiguousarray(rr_arr[core]).astype(s1_np),
            "xt": xTr,
            "wsrel": wsrelT,
            "wsroot": wsrootT,
            "wvrel": wvrel_packed,
            "wvroot": wvroot_packed,
            "bias": bias_t,
            "iota": iota_t,
            "ident": ident_t,
        })

    key = (meta["T1"], meta["T2"], meta["T3"], meta["NT"], meta["VQ"],
           meta["NQ"], meta["cols_off"], meta["rr_off"], cfg)
    nc = _get_program(key)
    kw = {}
    if PROFILE["on"]:
        kw = dict(trace=True, trace_cores=PROFILE["trace_cores"])
    res = run_bass_kernel_spmd(nc, in_maps, list(range(N_CORES)), **kw)
    PROFILE["last"] = res

    out_full = np.empty((NP_PAD, F), dtype=np.float32)
    for core in range(N_CORES):
        o = np.asarray(res.results[core]["out"]).astype(np.float32)
        base = core * NODES_PER_CORE
        for i in range(CHUNKS_PER_CORE):
            g_local = perms[core][i]
            out_full[base + g_local * P: base + (g_local + 1) * P] = \
                o[i * P:(i + 1) * P]
    return np.ascontiguousarray(
        out_full[:N_NODES].reshape(N_NODES, 4, H).astype(np.float32))
